# revision 10
# baseline (speedup 1.0000x reference)
"""Trainium2 Bass kernel for ConvReshapeBefore (im2col patch extraction).

Full problem: x (32, 64, 64, 64) f32 NHWC, kernel 3x3 stride 1 valid ->
out (62*62*32, 3, 3, 64) f32 where out[(r*62+c)*32 + b] = x[b, r:r+3, c:c+3, :].

Sharding: data-parallel over batch, 4 batches per core across 8 cores.

Per-core pipeline:
  1. load x shard -> SBUF xt[p = h + 64*(b%2), free = (b//2)*4096 + w*64 + k]
     (4 HWDGE DMAs: even batches on the sync ring -> partitions 0-63 /
     even SDMA engines, odd batches on the scalar ring -> partitions
     64-127 / odd engines, so the two rings drain concurrently)
  2. PE matmuls in transpose mode (exact 0/1 routing):
     psum[r, (w', k)] = sum_h Id[h, r+i] * xt[h, (c0+w')*64+k]
     for each (c-chunk u of 6, kernel-row i, batch b): 132 matmuls, N<=512
  3. DVE+ACT copies expand the j-overlap AND shift partitions up by 32:
     stage[32+r, c*2304 + b*576 + i*192 + j*64 + k] = psum[r, (c+j)*64+k]
     Each copy is split at the r=32 boundary (rows 0-31 -> partitions
     32-63, rows 32-61 -> partitions 64-93) because compute APs may not
     span >32 partitions starting from offset 32.
  4. SWDGE stores: per chunk one DMA of exactly 64 partitions 32-95
     (94-95 are pad) -> [[142848, 64], [1, csz*2304]].  64 partitions
     straddling the p=64 engine-group boundary at offset exactly 32
     engage all 16 SDMA engines (~360 GB/s measured; 62 partitions at
     offset 0 run at ~56 GB/s, other offsets ~190 GB/s).  out DRAM has
     2 trailing garbage rows the host slices off.
"""

import numpy as np

import concourse.bass as bass
import concourse.mybir as mybir
from concourse.ap import AP
from concourse.bass_utils import run_bass_kernel_spmd

# Full-problem constants (hardcoded per harness contract)
B, H, W, C = 32, 64, 64, 64
K = 3
R = H - K + 1  # 62
NCORES = 8
BS = B // NCORES  # 4

WC = W * C                    # 4096
ROW = 2 * WC                  # 8192 f32 per partition of xt
RUN = BS * K * K * C          # 2304 f32 per (r, c) output run
OUT_STRIDE_R = R * RUN        # 142848
CHUNKS = [(c0, min(6, R - c0)) for c0 in range(0, R, 6)]  # 11 chunks
NMM = len(CHUNKS) * K * BS    # 132 matmuls
BUF = 6 * RUN                 # f32 per stage buffer (ping-pong)
PS = 2 * BUF                  # stage partition stride (f32)
PSROW = 4096                  # psum f32 per partition (8 banks x 512)
POFF = 32                     # stage partition offset (stores at 32-95)
OUT_ROWS = 66                 # DRAM rows allocated (62 real + pad)


def _build_nc() -> bass.Bass:
    nc = bass.Bass(target_bir_lowering=False)
    x = nc.dram_tensor("x", [BS, H, W, C], mybir.dt.float32, kind="ExternalInput")
    out = nc.dram_tensor(
        "out", [OUT_ROWS * OUT_STRIDE_R], mybir.dt.float32, kind="ExternalOutput"
    )

    mms = [
        (u, i, b)
        for u in range(len(CHUNKS))
        for i in range(K)
        for b in range(BS)
    ]

    with (
        nc.sbuf_tensor("xt", [128, ROW], mybir.dt.float32) as xt,
        nc.sbuf_tensor("stage", [128, PS], mybir.dt.float32) as stage,
        nc.sbuf_tensor("iop", [128, 64], mybir.dt.float32) as iop,
        nc.sbuf_tensor("iof", [128, 64], mybir.dt.float32) as iof,
        nc.sbuf_tensor("ident", [128, 64], mybir.dt.float32) as ident,
        nc.psum_tensor("ps", [128, PSROW], mybir.dt.float32) as ps,
        nc.semaphore("l_e") as l_e,
        nc.semaphore("l_o") as l_o,
        nc.semaphore("isem") as isem,
        nc.semaphore("mm_sem") as mm_sem,
        nc.semaphore("cp0") as cp0,
        nc.semaphore("cp1") as cp1,
        nc.semaphore("st_e") as st_e,
        nc.semaphore("st_o") as st_o,
        nc.Block() as block,
    ):
        def copy_aps(n, r0, rn, poff):
            # rows [r0, r0+rn) of psum -> stage partitions [r0+poff, ...).
            # The (j, k) dims are merged into one 192-elem contiguous run;
            # the source c-stride (64) is smaller than the run, so reads
            # overlap -- that overlap IS the j-expansion.
            u, i, b = mms[n]
            c0, csz = CHUNKS[u]
            src = AP(
                ps,
                r0 * PSROW + (n % 8) * 512,
                [[PSROW, rn], [C, csz], [1, K * C]],
            )
            dst = AP(
                stage,
                (r0 + poff) * PS + (u % 2) * BUF + b * K * K * C + i * K * C,
                [[PS, rn], [RUN, csz], [1, K * C]],
            )
            return dst, src

        def load_aps(b):
            src = AP(x, b * H * WC, [[WC, H], [1, WC]])
            dst = AP(xt, (H * (b % 2)) * ROW + (b // 2) * WC, [[ROW, H], [1, WC]])
            return dst, src

        @block.sync
        def _(sync):
            for b in (0, 2):
                dst, src = load_aps(b)
                sync.dma_start(dst, src).then_inc(l_e, 16)

        @block.gpsimd
        def _(gp):
            gp.iota(
                AP(iop, 0, [[64, 128], [1, 64]]),
                [[0, 64]],
                channel_multiplier=1,
                allow_small_or_imprecise_dtypes=True,
            ).then_inc(isem, 1)
            gp.iota(
                AP(iof, 0, [[64, 64], [1, 64]]),
                [[1, 64]],
                channel_multiplier=0,
                allow_small_or_imprecise_dtypes=True,
            ).then_inc(isem, 1)
            gp.iota(
                AP(iof, 64 * 64, [[64, 64], [1, 64]]),
                [[1, 64]],
                base=64,
                channel_multiplier=0,
                allow_small_or_imprecise_dtypes=True,
            ).then_inc(isem, 1)
            for u, (c0, csz) in enumerate(CHUNKS):
                gp.wait_ge(cp0, 6 * (u + 1))
                gp.wait_ge(cp1, 6 * (u + 1))
                src = AP(stage, POFF * PS + (u % 2) * BUF, [[PS, 64], [1, csz * RUN]])
                dst = AP(out, c0 * RUN, [[OUT_STRIDE_R, 64], [1, csz * RUN]])
                gp.dma_start(dst, src).then_inc((st_e, st_o)[u % 2], 16)
            gp.wait_ge(st_e, 16 * 6)
            gp.wait_ge(st_o, 16 * 5)

        @block.vector
        def _(vec):
            vec.wait_ge(isem, 3)
            vec.tensor_tensor(
                AP(ident, 0, [[64, 128], [1, 64]]),
                AP(iop, 0, [[64, 128], [1, 64]]),
                AP(iof, 0, [[64, 128], [1, 64]]),
                mybir.AluOpType.is_equal,
            ).then_inc(isem, 1)
            for n in range(NMM):
                if n % 2 != 0:
                    continue
                u = mms[n][0]
                vec.wait_ge(mm_sem, n + 1)
                if u >= 2:
                    vec.wait_ge((st_e, st_o)[u % 2], 16 * (u // 2))
                dst, src = copy_aps(n, 0, 32, POFF)
                vec.tensor_copy(dst, src)
                dst, src = copy_aps(n, 32, 30, POFF)
                vec.tensor_copy(dst, src).then_inc(cp0, 1)

        @block.scalar
        def _(sc):
            for b in (1, 3):
                dst, src = load_aps(b)
                sc.dma_start(dst, src).then_inc(l_o, 16)
            for n in range(NMM):
                if n % 2 != 1:
                    continue
                u = mms[n][0]
                sc.wait_ge(mm_sem, n + 1)
                if u >= 2:
                    sc.wait_ge((st_e, st_o)[u % 2], 16 * (u // 2))
                dst, src = copy_aps(n, 0, 32, POFF)
                sc.copy(dst, src)
                dst, src = copy_aps(n, 32, 30, POFF)
                sc.copy(dst, src).then_inc(cp1, 1)

        @block.tensor
        def _(te):
            te.wait_ge(isem, 4)
            for n in range(NMM):
                u, i, b = mms[n]
                c0, csz = CHUNKS[u]
                if n < BS:
                    te.wait_ge((l_e, l_o)[b % 2], 16 * (b // 2 + 1))
                if n >= 8:
                    j = n - 8
                    te.wait_ge((cp0, cp1)[j % 2], j // 2 + 1)
                nfree = (csz + 2) * C
                out_ap = AP(ps, (n % 8) * 512, [[PSROW, R], [1, nfree]])
                lhsT = AP(ident, (b % 2) * H * 64 + i, [[64, 64], [1, R]])
                rhs = AP(
                    xt,
                    (b % 2) * H * ROW + (b // 2) * WC + c0 * C,
                    [[ROW, H], [C, csz + 2], [1, C]],
                )
                te.matmul(out_ap, lhsT, rhs).then_inc(mm_sem, 1)

    return nc


_NC = None


def _get_nc():
    global _NC
    if _NC is None:
        _NC = _build_nc()
    return _NC


def kernel(x: np.ndarray, **_run_kwargs) -> np.ndarray:
    assert x.shape == (B, H, W, C), x.shape
    nc = _get_nc()
    x = np.ascontiguousarray(x, dtype=np.float32)
    in_maps = [{"x": x[d * BS : (d + 1) * BS]} for d in range(NCORES)]
    res = run_bass_kernel_spmd(nc, in_maps, list(range(NCORES)), **_run_kwargs)
    outs = [
        res.results[d]["out"][: R * R * BS * K * K * C].reshape(R * R, BS, K, K, C)
        for d in range(NCORES)
    ]
    full = np.concatenate(outs, axis=1).reshape(R * R * B, K, K, C)
    if _run_kwargs:
        return full, res
    return full


# revision 11
# speedup vs baseline: 1.1779x; 1.1779x over previous
"""Trainium2 Bass kernel for ConvReshapeBefore (im2col patch extraction).

Full problem: x (32, 64, 64, 64) f32 NHWC, kernel 3x3 stride 1 valid ->
out (62*62*32, 3, 3, 64) f32 where out[(r*62+c)*32 + b] = x[b, r:r+3, c:c+3, :].

Sharding: data-parallel over batch, 4 batches per core across 8 cores.

Per-core pipeline:
  1. load x shard -> SBUF xt[p = h + 64*(b%2), free = (b//2)*4096 + w*64 + k]
     (even batches on the sync HWDGE ring -> partitions 0-63, odd batches
     on the scalar ring -> partitions 64-127; rings drain concurrently)
  2. PE matmuls (exact 0/1 routing): for c-chunk u, kernel-row i, batch b:
     psum[hf + r, (w', k)] = sum_h Id[h, r+i] * xt[h, (c0+w')*64+k]
     where hf = 64*(u%2): chunks alternate psum/stage partition halves.
  3. DVE+ACT copies expand the j-overlap (unsplit, 62 lanes):
     stage[hf+r, c*2304 + b*576 + i*192 + (jk)] = psum[hf+r, (c+j)*64+k]
     with the (j,k) dims merged into one 192-elem run whose source
     c-stride (64) overlaps -- the overlap IS the j-expansion.
  4. Stores: per chunk TWO 2D DMAs (rows 0-31 as 32 partitions, rows
     32-61 as 30) from the chunk's half.  Even chunks issue on the
     gpsimd SWDGE queue, odd chunks on the sync HWDGE queue: the two
     partition halves map to disjoint SDMA engine groups (~180 GB/s
     each), so the queues stream concurrently at ~350 GB/s aggregate.
     (One 62-partition dma runs at only ~56 GB/s; 64@32 is fast but
     requires padded DRAM rows.)
"""

import numpy as np

import concourse.bass as bass
import concourse.mybir as mybir
from concourse.ap import AP
from concourse.bass_utils import run_bass_kernel_spmd

# Full-problem constants (hardcoded per harness contract)
B, H, W, C = 32, 64, 64, 64
K = 3
R = H - K + 1  # 62
NCORES = 8
BS = B // NCORES  # 4

WC = W * C                    # 4096
ROW = 2 * WC                  # 8192 f32 per partition of xt
RUN = BS * K * K * C          # 2304 f32 per (r, c) output run
OUT_STRIDE_R = R * RUN        # 142848
CHUNKS = [(c0, min(6, R - c0)) for c0 in range(0, R, 6)]  # 11 chunks
NCH = len(CHUNKS)
NMM = NCH * K * BS            # 132 matmuls
BUF = 6 * RUN                 # f32 per stage buffer (ping-pong per half)
PS = 2 * BUF                  # stage partition stride (f32)
PSROW = 4096                  # psum f32 per partition (8 banks x 512)


def _build_nc() -> bass.Bass:
    nc = bass.Bass(target_bir_lowering=False)
    x = nc.dram_tensor("x", [BS, H, W, C], mybir.dt.float32, kind="ExternalInput")
    out = nc.dram_tensor(
        "out", [R * R * BS, K, K, C], mybir.dt.float32, kind="ExternalOutput"
    )

    mms = [
        (u, i, b)
        for u in range(NCH)
        for i in range(K)
        for b in range(BS)
    ]

    # chunk u lives on partition half 64*(u%2), stage buffer (u//2)%2
    def half(u):
        return 64 * (u % 2)

    def sbuf(u):
        return ((u // 2) % 2) * BUF

    with (
        nc.sbuf_tensor("xt", [128, ROW], mybir.dt.float32) as xt,
        nc.sbuf_tensor("stage", [128, PS], mybir.dt.float32) as stage,
        nc.sbuf_tensor("iop", [128, 64], mybir.dt.float32) as iop,
        nc.sbuf_tensor("iof", [128, 64], mybir.dt.float32) as iof,
        nc.sbuf_tensor("ident", [128, 64], mybir.dt.float32) as ident,
        nc.psum_tensor("ps", [128, PSROW], mybir.dt.float32) as ps,
        nc.semaphore("l_e") as l_e,
        nc.semaphore("l_o") as l_o,
        nc.semaphore("isem") as isem,
        nc.semaphore("mm_sem") as mm_sem,
        nc.semaphore("cp0") as cp0,
        nc.semaphore("cp1") as cp1,
        nc.semaphore("st0") as st0,
        nc.semaphore("st1") as st1,
        nc.semaphore("st2") as st2,
        nc.semaphore("st3") as st3,
        nc.Block() as block,
    ):
        sts = (st0, st1, st2, st3)
        # total st incs per u%4 class: 32 per chunk
        st_tot = [32 * len([u for u in range(NCH) if u % 4 == q]) for q in range(4)]

        def copy_aps(n):
            u, i, b = mms[n]
            c0, csz = CHUNKS[u]
            src = AP(
                ps,
                half(u) * PSROW + (n % 8) * 512,
                [[PSROW, R], [C, csz], [1, K * C]],
            )
            dst = AP(
                stage,
                half(u) * PS + sbuf(u) + b * K * K * C + i * K * C,
                [[PS, R], [RUN, csz], [1, K * C]],
            )
            return dst, src

        def store_aps(u, lo):
            # lo: rows 0-31 (32 partitions); else rows 32-61 (30 partitions)
            c0, csz = CHUNKS[u]
            p0, np_ = (0, 32) if lo else (32, 30)
            src = AP(
                stage,
                (half(u) + p0) * PS + sbuf(u),
                [[PS, np_], [1, csz * RUN]],
            )
            dst = AP(
                out,
                c0 * RUN + p0 * OUT_STRIDE_R,
                [[OUT_STRIDE_R, np_], [1, csz * RUN]],
            )
            return dst, src

        def load_aps(b):
            src = AP(x, b * H * WC, [[WC, H], [1, WC]])
            dst = AP(xt, (H * (b % 2)) * ROW + (b // 2) * WC, [[ROW, H], [1, WC]])
            return dst, src

        @block.sync
        def _(sync):
            for b in (0, 2):
                dst, src = load_aps(b)
                sync.dma_start(dst, src).then_inc(l_e, 16)
            for u in range(1, NCH, 2):
                sync.wait_ge(cp0, 6 * (u + 1))
                sync.wait_ge(cp1, 6 * (u + 1))
                for lo in (True, False):
                    dst, src = store_aps(u, lo)
                    sync.dma_start(dst, src).then_inc(sts[u % 4], 16)
            for q in range(4):
                sync.wait_ge(sts[q], st_tot[q])

        @block.gpsimd
        def _(gp):
            gp.iota(
                AP(iop, 0, [[64, 128], [1, 64]]),
                [[0, 64]],
                channel_multiplier=1,
                allow_small_or_imprecise_dtypes=True,
            ).then_inc(isem, 1)
            gp.iota(
                AP(iof, 0, [[64, 64], [1, 64]]),
                [[1, 64]],
                channel_multiplier=0,
                allow_small_or_imprecise_dtypes=True,
            ).then_inc(isem, 1)
            gp.iota(
                AP(iof, 64 * 64, [[64, 64], [1, 64]]),
                [[1, 64]],
                base=64,
                channel_multiplier=0,
                allow_small_or_imprecise_dtypes=True,
            ).then_inc(isem, 1)
            for u in range(0, NCH, 2):
                gp.wait_ge(cp0, 6 * (u + 1))
                gp.wait_ge(cp1, 6 * (u + 1))
                for lo in (True, False):
                    dst, src = store_aps(u, lo)
                    gp.dma_start(dst, src).then_inc(sts[u % 4], 16)
            for q in range(4):
                gp.wait_ge(sts[q], st_tot[q])

        @block.vector
        def _(vec):
            vec.wait_ge(isem, 3)
            vec.tensor_tensor(
                AP(ident, 0, [[64, 128], [1, 64]]),
                AP(iop, 0, [[64, 128], [1, 64]]),
                AP(iof, 0, [[64, 128], [1, 64]]),
                mybir.AluOpType.is_equal,
            ).then_inc(isem, 1)
            for n in range(NMM):
                if n % 2 != 0:
                    continue
                u = mms[n][0]
                vec.wait_ge(mm_sem, n + 1)
                if u >= 4:
                    vec.wait_ge(sts[u % 4], 32 * (u // 4))
                dst, src = copy_aps(n)
                vec.tensor_copy(dst, src).then_inc(cp0, 1)

        @block.scalar
        def _(sc):
            for b in (1, 3):
                dst, src = load_aps(b)
                sc.dma_start(dst, src).then_inc(l_o, 16)
            for n in range(NMM):
                if n % 2 != 1:
                    continue
                u = mms[n][0]
                sc.wait_ge(mm_sem, n + 1)
                if u >= 4:
                    sc.wait_ge(sts[u % 4], 32 * (u // 4))
                dst, src = copy_aps(n)
                sc.copy(dst, src).then_inc(cp1, 1)

        @block.tensor
        def _(te):
            te.wait_ge(isem, 4)
            for n in range(NMM):
                u, i, b = mms[n]
                c0, csz = CHUNKS[u]
                if n < BS:
                    te.wait_ge((l_e, l_o)[b % 2], 16 * (b // 2 + 1))
                if n >= 8:
                    j = n - 8
                    te.wait_ge((cp0, cp1)[j % 2], j // 2 + 1)
                nfree = (csz + 2) * C
                out_ap = AP(
                    ps, half(u) * PSROW + (n % 8) * 512, [[PSROW, R], [1, nfree]]
                )
                lhsT = AP(ident, (b % 2) * H * 64 + i, [[64, 64], [1, R]])
                rhs = AP(
                    xt,
                    (b % 2) * H * ROW + (b // 2) * WC + c0 * C,
                    [[ROW, H], [C, csz + 2], [1, C]],
                )
                te.matmul(out_ap, lhsT, rhs).then_inc(mm_sem, 1)

    return nc


_NC = None


def _get_nc():
    global _NC
    if _NC is None:
        _NC = _build_nc()
    return _NC


def kernel(x: np.ndarray, **_run_kwargs) -> np.ndarray:
    assert x.shape == (B, H, W, C), x.shape
    nc = _get_nc()
    x = np.ascontiguousarray(x, dtype=np.float32)
    in_maps = [{"x": x[d * BS : (d + 1) * BS]} for d in range(NCORES)]
    res = run_bass_kernel_spmd(nc, in_maps, list(range(NCORES)), **_run_kwargs)
    outs = [res.results[d]["out"].reshape(R * R, BS, K, K, C) for d in range(NCORES)]
    full = np.concatenate(outs, axis=1).reshape(R * R * B, K, K, C)
    if _run_kwargs:
        return full, res
    return full
